# revision 22
# baseline (speedup 1.0000x reference)
"""BertSelfAttention Trainium2 Bass kernel.

Full (unsharded) inputs in, full output out. Internally shards across 8
NeuronCores as (batch b, head-group g): core c handles batch c//2 and
heads [6*(c%2), 6*(c%2)+6) of the 12 heads.

Per-core program (Tile framework):
  A) load hs[b], W/bias slices, mask[b]; PE-transpose to put the
     contraction dim on partitions (hsT [d,q], WT [d,out], maskT [k,1]).
  B) QT/KT [128=2 heads, 2048] via matmul; V [k, dh] directly (bias via
     rank-1 ones-row matmul); ones column appended per head for row-sums.
  C) per head, per q-chunk of 1024: flash-style loop over 16 k-tiles:
     scoresT [k-tile, q-chunk] in PSUM -> one ACT instruction does
     exp(0.125*s + mask_k) (scale folds 1/sqrt(64), per-partition bias
     folds the additive attention mask) -> probsT bf16 -> PV accumulates
     ctxT [65, q-chunk] in PSUM (row 64 = softmax denominator).
     Tail: PE-transpose ctxT -> [q, 65], DVE reciprocal + scale, DMA out.
"""

import os
import sys

sys.path.insert(0, "/opt/trn_rl_repo")

import numpy as np

B, S, D = 4, 2048, 768
H, DH = 12, 64
NCORES = 8
HPC = 6          # heads per core
GSZ = HPC * DH   # 384 output dims per core
P = 128
ND = D // P      # 6 d-tiles
NT = S // P      # 16 k-tiles
QC = 1024        # q-chunk
MMN = 512        # matmul free dim per instruction (fp32 limit)
MMN2 = 1024      # free dim for bf16 scores/PV matmuls

_cache = {}


def _build(mm_dt_name: str, loop_n: int = 0):
    key = (mm_dt_name, loop_n)
    if key in _cache:
        return _cache[key]

    import concourse.bass as bass
    import concourse.bacc as bacc
    import concourse.mybir as mybir
    from concourse import tile
    from concourse.masks import make_identity

    f32 = mybir.dt.float32
    mm_dt = getattr(mybir.dt, mm_dt_name)
    AF = mybir.ActivationFunctionType

    nc = bacc.Bacc("TRN2", target_bir_lowering=False, debug=False,
                   num_devices=NCORES)

    hs_d = nc.dram_tensor("hs", [S, D], f32, kind="ExternalInput")
    w_d = {p: nc.dram_tensor(f"w{p}", [GSZ, D], f32, kind="ExternalInput")
           for p in "qkv"}
    bias_d = nc.dram_tensor("bias", [3, GSZ], f32, kind="ExternalInput")
    mask_d = nc.dram_tensor("mask", [NT, P], f32, kind="ExternalInput")
    out_d = nc.dram_tensor("out", [S, GSZ], f32, kind="ExternalOutput")

    with tile.TileContext(nc) as tc:
        with tc.tile_pool(name="const", bufs=1) as const_pool, \
             tc.tile_pool(name="persist", bufs=1) as pers:

            ident = const_pool.tile([P, P], f32)
            make_identity(nc, ident[:])
            ident_mm = const_pool.tile([P, P], mm_dt)
            make_identity(nc, ident_mm[:])

            # ---- persistent SBUF tensors ----
            hsT = pers.tile([P, ND, S], mm_dt, tag="hsT")       # [d%128, dtile, q]
            wT = {p: pers.tile([P, ND, GSZ], mm_dt, tag=f"wT{p}", name=f"wT{p}")
                  for p in "qkv"}
            maskT = pers.tile([P, NT], f32, tag="maskT")        # [k%128, ktile]
            biasT = pers.tile([P, 6], f32, tag="biasT")         # [dim%128, pair*2+proj(q,k)]
            qT = pers.tile([P, 3, S], mm_dt, tag="qT")          # [2*dh, pair, q]
            kT = pers.tile([P, 3, S], mm_dt, tag="kT")
            vsb = pers.tile([P, NT, HPC * (DH + 1)], mm_dt, tag="vsb")
            bvrow = pers.tile([1, GSZ], mm_dt, tag="bvrow")
            onesrow = pers.tile([1, P], mm_dt, tag="onesrow")

            nc.vector.memset(vsb[:], 1.0)     # ones columns; v dims overwritten
            nc.vector.memset(onesrow[:], 1.0)

            import contextlib
            loop_cm = (tc.For_i(0, loop_n, 1,
                                hint_engines=(mybir.EngineType.PE,
                                              mybir.EngineType.Activation,
                                              mybir.EngineType.DVE,
                                              mybir.EngineType.SP))
                       if loop_n else contextlib.nullcontext())
            with loop_cm:
                # ================= Phase A: loads + transposes =================
                # single psum pool ("saps", 2x2-bank slots) serves phase A
                # transposes, phase B V-projection AND phase C score tiles:
                # sharing slots avoids the cross-phase bank-reuse WAR wall
                # that serialized A/B against C's start.
                with tc.tile_pool(name="stage", bufs=6) as stage, \
                     tc.tile_pool(name="saps", bufs=2, space="PSUM") as saps, \
                     tc.tile_pool(name="ctxps", bufs=2, space="PSUM") as ctxps, \
                     tc.tile_pool(name="tpps", bufs=2, space="PSUM") as tpps, \
                     tc.tile_pool(name="probs", bufs=6) as probs_pool, \
                     tc.tile_pool(name="tailsb", bufs=2) as tailsb, \
                     tc.tile_pool(name="outsb", bufs=4) as outsb:

                    # mask [NT, P] -> maskT [P, NT]
                    mstage = stage.tile([NT, P], f32, tag="mstage")
                    nc.sync.dma_start(mstage[:], mask_d[:])
                    mps = saps.tile([P, NT], f32, tag="sa", name="mps")
                    nc.tensor.transpose(mps[:], mstage[:], ident[:NT, :NT])
                    nc.vector.tensor_copy(maskT[:], mps[:])

                    # bias [3, GSZ] -> biasT [P, proj(q,k), pair]; bv -> bvrow
                    bstage = stage.tile([3, GSZ], f32, tag="bstage")
                    nc.sync.dma_start(bstage[:], bias_d[:])
                    for pp in range(3):
                        bps = saps.tile([P, 3], f32, tag="sa", name="bps")
                        nc.tensor.transpose(bps[:], bstage[:, pp * P:(pp + 1) * P],
                                            ident[:3, :3])
                        nc.vector.tensor_copy(biasT[:, pp * 2:pp * 2 + 2], bps[:, 0:2])
                    bvstage = stage.tile([1, GSZ], f32, tag="bvstage")
                    nc.sync.dma_start(bvstage[:], bias_d[2:3, :])
                    nc.vector.tensor_copy(bvrow[0:1, :], bvstage[0:1, :])

                    # hs -> hsT (cast to mm_dt on one batched eviction per tile)
                    for t in range(NT):
                        hstage = stage.tile([P, D], f32, tag="hstage")
                        nc.sync.dma_start(hstage[:], hs_d[t * P:(t + 1) * P, :])
                        ps = saps.tile([P, D], f32, tag="sa", name="trb")
                        for d in range(ND):
                            nc.tensor.transpose(ps[:, d * P:(d + 1) * P],
                                                hstage[:, d * P:(d + 1) * P],
                                                ident[:])
                        nc.vector.tensor_copy(
                            hsT[:, :, t * P:(t + 1) * P],
                            ps[:].rearrange("p (d c) -> p d c", c=P))

                    # W slices -> wT
                    for p in "qkv":
                        for r in range(GSZ // P):  # 3 row-tiles of 128 outdims
                            wstage = stage.tile([P, D], f32, tag="wstage")
                            nc.sync.dma_start(wstage[:], w_d[p][r * P:(r + 1) * P, :])
                            ps = saps.tile([P, D], f32, tag="sa", name="trb")
                            for d in range(ND):
                                nc.tensor.transpose(ps[:, d * P:(d + 1) * P],
                                                    wstage[:, d * P:(d + 1) * P],
                                                    ident[:])
                            nc.vector.tensor_copy(
                                wT[p][:, :, r * P:(r + 1) * P],
                                ps[:].rearrange("p (d c) -> p d c", c=P))

                    # ================= Phase B: V projection =================
                    if True:
                        # V [k, dh] per k-tile (+ bias via rank-1 ones x bv)
                        for t in range(NT):
                            ps = saps.tile([P, GSZ], f32, tag="sa", name="vp")
                            for d in range(ND):
                                nc.tensor.matmul(ps[:], hsT[:, d, t * P:(t + 1) * P],
                                                 wT["v"][:, d, :],
                                                 start=(d == 0), stop=False)
                            nc.tensor.matmul(ps[:], onesrow[0:1, :],
                                             bvrow[0:1, :], start=False, stop=True)
                            nc.vector.tensor_copy(
                                vsb[:, t, :].rearrange("p (h c) -> p h c", c=DH + 1)[:, :, 0:DH],
                                ps[:].rearrange("p (h c) -> p h c", c=DH))

                    # ================= Phase C: attention per head =================
                    # QC2=512: scores psum tiles are 1 bank each -> 4 concurrent
                    # slots allow both heads of a pair to issue score matmuls
                    # adjacently (row-group packed on PE).
                    if True:
                        QC2 = 512

                        def emit_qk_chain(pp3, pi, pname, ch):
                            dst = qT if pname == "q" else kT
                            qkp = saps.tile([P, QC2], f32, tag="sa", name="qkp")
                            for d in range(ND):
                                nc.tensor.matmul(
                                    qkp[:],
                                    wT[pname][:, d, pp3 * P:(pp3 + 1) * P],
                                    hsT[:, d, ch * QC2:(ch + 1) * QC2],
                                    start=(d == 0), stop=(d == ND - 1))
                            nc.vector.tensor_scalar_add(
                                dst[:, pp3, ch * QC2:(ch + 1) * QC2],
                                qkp[:], biasT[:, pp3 * 2 + pi:pp3 * 2 + pi + 1])

                        for pp3 in range(3):
                            # QT / KT projections: pair 0's emitted here; later
                            # pairs' chains are split into 2-chain blocks at the
                            # previous pair's q-chunk boundaries (below), small
                            # enough for the sa double-buffer lookahead to absorb
                            # instead of one 21us pair-boundary stall.
                            if pp3 == 0:
                                for pi, pname in enumerate(("q", "k")):
                                    for ch in range(S // QC2):
                                        emit_qk_chain(pp3, pi, pname, ch)

                            hA, hB = 2 * pp3, 2 * pp3 + 1
                            for qc in range(S // QC2):
                                ctxs = {}
                                for h in (hA, hB):
                                    ctxs[h] = ctxps.tile([DH + 1, QC2], f32, tag="ctx",
                                                         name=f"ctx{h}")
                                for t in range(NT):
                                    # both heads' scores land in ONE [128, 1024]
                                    # psum tile (halves = different banks, written
                                    # by row-group-packed matmuls) so a single
                                    # ACT instruction exps both heads at once
                                    sa = saps.tile([P, 2 * QC2], f32, tag="sa",
                                                   name="sa")
                                    for i, h in enumerate((hA, hB)):
                                        base = (h % 2) * DH
                                        nc.tensor.matmul(
                                            sa[:, i * QC2:(i + 1) * QC2],
                                            kT[base:base + DH, pp3, t * P:(t + 1) * P],
                                            qT[base:base + DH, pp3,
                                               qc * QC2:(qc + 1) * QC2],
                                            start=True, stop=True)
                                    pr = probs_pool.tile([P, 2 * QC2], mm_dt,
                                                         tag="pr", name="pr")
                                    nc.scalar.activation(pr[:], sa[:], AF.Exp,
                                                         bias=maskT[:, t:t + 1],
                                                         scale=0.125)
                                    for i, h in enumerate((hA, hB)):
                                        nc.tensor.matmul(
                                            ctxs[h][:],
                                            vsb[:, t, h * (DH + 1):(h + 1) * (DH + 1)],
                                            pr[:, i * QC2:(i + 1) * QC2],
                                            start=(t == 0), stop=(t == NT - 1))
                                # tail: normalize + transpose + store
                                for h in (hA, hB):
                                    ctxu = tailsb.tile([DH + 1, QC2], f32, tag="ctxu",
                                                       name=f"ctxu{h}")
                                    nc.vector.tensor_copy(ctxu[:], ctxs[h][:])
                                    for s2 in range(QC2 // P):
                                        tp = tpps.tile([P, DH + 1], f32, tag="tp")
                                        nc.tensor.transpose(
                                            tp[:], ctxu[:, s2 * P:(s2 + 1) * P],
                                            ident[:DH + 1, :DH + 1])
                                        rcp = outsb.tile([P, 1], f32, tag="rcp")
                                        nc.vector.reciprocal(rcp[:], tp[:, DH:DH + 1])
                                        ot = outsb.tile([P, DH], f32, tag="ot")
                                        nc.vector.tensor_scalar_mul(ot[:], tp[:, 0:DH],
                                                                    rcp[:])
                                        q0 = qc * QC2 + s2 * P
                                        nc.sync.dma_start(
                                            out_d[q0:q0 + P, h * DH:(h + 1) * DH],
                                            ot[:])
                                # two of the NEXT pair's Q/K chains per chunk
                                if pp3 < 2:
                                    for j in (2 * qc, 2 * qc + 1):
                                        pi, ch = (0, j) if j < 4 else (1, j - 4)
                                        emit_qk_chain(pp3 + 1, pi,
                                                      "q" if pi == 0 else "k", ch)

    nc.compile()
    _cache[key] = nc
    return nc


def _in_maps(hidden_states, attention_mask, Wq, bq, Wk, bk, Wv, bv):
    maps = []
    for c in range(NCORES):
        b, g = c // 2, c % 2
        sl = slice(g * GSZ, (g + 1) * GSZ)
        maps.append({
            "hs": np.ascontiguousarray(hidden_states[b], dtype=np.float32),
            "wq": np.ascontiguousarray(Wq[sl], dtype=np.float32),
            "wk": np.ascontiguousarray(Wk[sl], dtype=np.float32),
            "wv": np.ascontiguousarray(Wv[sl], dtype=np.float32),
            "bias": np.ascontiguousarray(
                np.stack([bq[sl], bk[sl], bv[sl]]), dtype=np.float32),
            "mask": np.ascontiguousarray(
                attention_mask[b].reshape(NT, P), dtype=np.float32),
        })
    return maps


def kernel(hidden_states, attention_mask, Wq, bq, Wk, bk, Wv, bv,
           _trace=False, _tmpdir=None):
    from concourse.bass_utils import run_bass_kernel_spmd

    nc = _build(os.environ.get("BERT_MM_DT", "bfloat16"))
    maps = _in_maps(np.asarray(hidden_states), np.asarray(attention_mask),
                    np.asarray(Wq), np.asarray(bq), np.asarray(Wk),
                    np.asarray(bk), np.asarray(Wv), np.asarray(bv))
    res = run_bass_kernel_spmd(nc, maps, core_ids=list(range(NCORES)),
                               trace=_trace, tmpdir=_tmpdir)
    out = np.empty((B, S, D), dtype=np.float32)
    for c in range(NCORES):
        b, g = c // 2, c % 2
        out[b, :, g * GSZ:(g + 1) * GSZ] = res.results[c]["out"]
    kernel.last_results = res
    return out
